# revision 14
# baseline (speedup 1.0000x reference)
# Trainium2 Bass kernel for nn_BinLinearEval:
#   out[b, o] = (round(x @ W.T + bias) * sign >= 0) ? 1.0 : 0.0
#
# Math folding (exact because bias is integer-valued and sign in {-1,+1}):
#   out = 1  iff  sign*(dot + bias) >= -0.5
#       = 1  iff  dot' >= thr_o      where dot' = x @ (sign.T*W).T  (W' still
#         ternary) and thr_o = -sign_o*bias_o - 0.5.
#
# Precision: x is shipped as an e4m3 hi + e4m3 residual*64 pair (2 B/elem)
# and BOTH passes run as fp8 DoubleRow matmuls. ~1700 threshold flips of
# 16.7M (rel err ~0.014 vs the 2e-2 gate).
#
# Measured facts this schedule is built on (NTFF traces):
#  - At 8-core load the chip sits in P0: PE clock ~2.0 GHz, so a DR FD=512
#    matmul stream paces at exactly 259 ns/MM (216 ns single-core). The
#    256-MM stream is a hard 66.3 us floor; LDWEIGHTS fully hides in the
#    pull-ahead window at any weight-reuse pattern, so no LDW amortization
#    is needed.
#  - The framework preamble ends ~6.4 us; first DMA bytes move ~8.7-9 us.
#    Both HWDGE rings share the 16 SDMA engines per 4KB packet, so each
#    ring sustains ~175 GB/s while both are busy (~350 aggregate = HBM cap).
#  - Receipts (sem>=16) land ~50 ns after transfer-done; what matters is
#    pure need-ordering of the two ring FIFOs.
# Schedule: warmup MMs run on memset tiles (no DMA dependency) from ~7 us
# so HAM un-throttles before real data lands; the first 3 groups and w8
# are split across both rings in need order; outs are merged per group
# (1 KB/partition) and alternate rings.

import os
from contextlib import ExitStack

import numpy as np
import ml_dtypes

BATCH, IN_F, OUT_F = 65536, 1024, 256
N_CORES = 8
B_CORE = BATCH // N_CORES  # 8192
P = 128
KC = IN_F // P             # 8 k-chunks of 128
NCH = KC                   # 8 DoubleRow chunk-steps: 4 hi + 4 lo, 256-contract each
OC = OUT_F // P            # 2 out-channel chunks
GRP = 512                  # batch tile (= max DR matmul moving dim / 2)
N_GROUPS = B_CORE // GRP   # 16
N_WARM = 10                # dummy MMs spanning ~4.3 us of PE-busy before data

_CACHE = {}


def _build():
    """Build (and cache) the Bass module. Returns the compiled nc."""
    if "nc" in _CACHE:
        return _CACHE["nc"]

    import concourse.bacc as bacc
    import concourse.mybir as mybir
    import concourse.tile as tile

    nc = bacc.Bacc(
        "TRN2",
        target_bir_lowering=False,
        debug=False,
        num_devices=N_CORES,
    )

    f32 = mybir.dt.float32
    f8 = mybir.dt.float8e4
    DR = mybir.MatmulPerfMode.DoubleRow

    # x8 chunk layout: [P, group, chunk(0:4 hi, 4:8 lo), j, GRP] where the
    # DoubleRow pair (chunk c, j) covers global k = (c%4)*256 + j*128 + p
    x8_d = nc.dram_tensor(
        "x8", [P, N_GROUPS, NCH, 2, GRP], f8, kind="ExternalInput"
    ).ap()
    # weights split by oc so each half is one contiguous 2KB/partition DMA
    w8_d = nc.dram_tensor("w8", [P, OC, NCH, 2, P], f8, kind="ExternalInput").ap()
    thr_d = nc.dram_tensor("thr", [P, OC], f32, kind="ExternalInput").ap()
    out_d = nc.dram_tensor(
        "out", [P, N_GROUPS, OC, GRP], f8, kind="ExternalOutput"
    ).ap()

    with tile.TileContext(nc) as tc, ExitStack() as ctx:
        const = ctx.enter_context(tc.tile_pool(name="const", bufs=1))
        io = ctx.enter_context(tc.tile_pool(name="io", bufs=1))
        outp = ctx.enter_context(tc.tile_pool(name="outp", bufs=1))
        psum = ctx.enter_context(tc.tile_pool(name="psum", bufs=8, space="PSUM"))

        w8_sb = const.tile([P, OC, NCH, 2, P], f8)
        thr_sb = const.tile([P, OC], f32)
        warm_x = const.tile([P, 2, GRP], f8)

        xt = {}
        for g in range(N_GROUPS):
            xt[g] = io.tile([P, NCH, 2, GRP], f8, name=f"x{g}", bufs=1)

        # warmup operand comes from one memset, not DMA, so the PE can
        # start burning its HAM ramp right after the preamble barrier
        nc.vector.memset(warm_x, 0.25)

        # ── DMA triggers ──
        # Both HWDGE rings share the 16 SDMA engines per-packet, ~185 GB/s
        # each while both are busy. Every group is split hi/lo across the
        # two rings in lockstep so group k completes ~2.7k us after the
        # first bytes — always ahead of the PE's 3.46 us/group consumption.
        # thr's 8-byte-per-partition descriptors would waste ring turns at
        # the worst time, so it rides the idle gpsimd SWDGE path instead.
        # Ring start times vary per core/run by +-1.5 us, so the items the
        # first matmuls need (w8oc0, g0hi, g0lo, g1hi) are themselves split
        # across BOTH rings — the stream start depends on max(ring starts)
        # + ~2.1 us instead of one ring delivering 0.75 MB alone.
        H = NCH // 2
        Q = NCH // 4
        # both w8 halves first, one per ring (the interleaved stream needs
        # oc0-c0 AND oc1-c0 immediately); then g0/g1 in quarters
        # alternating rings, g2+ as hi/lo halves in parallel
        nc.sync.dma_start(out=w8_sb[:, 0], in_=w8_d[:, 0])
        nc.scalar.dma_start(out=w8_sb[:, 1], in_=w8_d[:, 1])
        for g in (0, 1):
            r = g % 2
            engs = [nc.sync, nc.scalar]
            for q in range(4):
                e = engs[(q + r) % 2]
                e.dma_start(
                    out=xt[g][:, 2 * q : 2 * q + 2], in_=x8_d[:, g, 2 * q : 2 * q + 2]
                )
        for g in range(2, N_GROUPS):
            nc.sync.dma_start(out=xt[g][:, :H], in_=x8_d[:, g, :H])
            nc.scalar.dma_start(out=xt[g][:, H:], in_=x8_d[:, g, H:])
            if g == 4:
                # thr's 8B-per-partition descriptors waste ~3 us of ring
                # turns, so it goes late; the first epilogue only becomes
                # psum-critical at ~25 us
                nc.scalar.dma_start(out=thr_sb, in_=thr_d)

        # ── PE warmup: data-independent DR MMs at cold pace (~0.43-0.52
        # us each) spanning ~4.3 us so HAM reaches K=8/8 before the first
        # real matmul. psum never read; slots recycle into the pool.
        wps = [psum.tile([P, GRP], f32, name="ps") for _ in range(2)]
        for i in range(N_WARM):
            nc.tensor.matmul(
                wps[i % 2], warm_x[:, :, :P], warm_x,
                start=True, stop=True, perf_mode=DR,
            )

        # ── main stream: 16 groups x 8 chunk-steps x 2 oc-passes ──
        # The oc passes are interleaved per chunk so fresh-x demand is a
        # steady ~296 GB/s (one 128KB chunk per 2 MMs) instead of 2x-supply
        # bursts during each oc0 pass — the rings then never fall behind.
        # The LAST group runs its oc passes sequentially so the oc0
        # epilogue + out drain ~1.7 us before the end (shorter tail), with
        # its out split per-oc across the two then-idle rings.
        for g in range(N_GROUPS):
            ob = outp.tile([P, OC, GRP], f8, name=f"ob{g}", bufs=1)
            last = g == N_GROUPS - 1
            if not last:
                pss = [psum.tile([P, GRP], f32, name="ps") for _ in range(OC)]
                for c in range(NCH):
                    for oc in range(OC):
                        nc.tensor.matmul(
                            pss[oc],
                            w8_sb[:, oc, c],
                            xt[g][:, c],
                            start=(c == 0),
                            stop=(c == NCH - 1),
                            perf_mode=DR,
                        )
                for oc in range(OC):
                    nc.vector.tensor_scalar(
                        ob[:, oc],
                        pss[oc],
                        thr_sb[:, oc : oc + 1],
                        None,
                        mybir.AluOpType.is_ge,
                    )
                eng = nc.sync if g % 2 else nc.scalar
                eng.dma_start(out=out_d[:, g], in_=ob)
            else:
                for oc in range(OC):
                    ps = psum.tile([P, GRP], f32, name="ps")
                    for c in range(NCH):
                        nc.tensor.matmul(
                            ps,
                            w8_sb[:, oc, c],
                            xt[g][:, c],
                            start=(c == 0),
                            stop=(c == NCH - 1),
                            perf_mode=DR,
                        )
                    nc.vector.tensor_scalar(
                        ob[:, oc],
                        ps,
                        thr_sb[:, oc : oc + 1],
                        None,
                        mybir.AluOpType.is_ge,
                    )
                    eng = nc.sync if oc == 0 else nc.scalar
                    eng.dma_start(out=out_d[:, g, oc], in_=ob[:, oc])

    nc.compile()
    _CACHE["nc"] = nc
    return nc


def _prep_inputs(x, weight, bias, sign):
    """Host-side prep: fold sign into weights, build thresholds, split x into
    an e4m3 hi + e4m3 residual*64 pair in DoubleRow-interleaved layout."""
    f8np = ml_dtypes.float8_e4m3fn
    x = np.asarray(x, dtype=np.float32)
    weight = np.asarray(weight, dtype=np.float32)
    bias = np.asarray(bias, dtype=np.float32)
    sign = np.asarray(sign, dtype=np.float32).reshape(1, OUT_F)

    wp = sign.T * weight                      # [OUT_F, IN_F], ternary
    thr = (-sign[0] * bias - np.float32(0.5)).astype(np.float32)  # [OUT_F]
    thr2 = np.ascontiguousarray(thr.reshape(OC, P).T)  # [P, OC]

    # weights: [P, oc, chunk, j, 128]; chunks 0:4 = W' (ternary, exact in
    # e4m3), 4:8 = W'/64 (+-2^-6, exact in e4m3)
    wT = wp.T  # [IN_F, OUT_F]
    whi = wT.reshape(NCH // 2, 2, P, OUT_F).transpose(2, 0, 1, 3)
    wlo = (wT * np.float32(1.0 / 64.0)).reshape(NCH // 2, 2, P, OUT_F).transpose(
        2, 0, 1, 3
    )
    w8 = np.concatenate([whi, wlo], axis=1)   # [P, NCH, 2, OUT_F]
    w8 = np.ascontiguousarray(
        w8.reshape(P, NCH, 2, OC, P).transpose(0, 3, 1, 2, 4)
    ).astype(f8np)                            # [P, OC, NCH, 2, P]

    xhi8 = x.astype(f8np)
    xlo8 = ((x - xhi8.astype(np.float32)) * np.float32(64.0)).astype(f8np)

    in_maps = []
    for c in range(N_CORES):
        sl = slice(c * B_CORE, (c + 1) * B_CORE)
        hi = xhi8[sl].reshape(N_GROUPS, GRP, NCH // 2, 2, P).transpose(
            4, 0, 2, 3, 1
        )                                      # [P, g, 4, 2, GRP]
        lo = xlo8[sl].reshape(N_GROUPS, GRP, NCH // 2, 2, P).transpose(
            4, 0, 2, 3, 1
        )
        x8 = np.ascontiguousarray(np.concatenate([hi, lo], axis=2))
        in_maps.append({"x8": x8, "w8": w8, "thr": thr2})
    return in_maps


def _assemble(results):
    """[core][P, N_GROUPS, OC, GRP] fp8 -> [BATCH, OUT_F] fp32"""
    parts = []
    for r in results:
        a = (
            np.asarray(r["out"])
            .view(ml_dtypes.float8_e4m3fn)
            .astype(np.float32)
            .reshape(P, N_GROUPS, OC, GRP)
        )
        # out[b, o]: b = g*GRP + col, o = oc*P + p
        parts.append(
            a.transpose(1, 3, 2, 0).reshape(B_CORE, OUT_F)
        )
    return np.ascontiguousarray(np.concatenate(parts, axis=0))


def run(x, weight, bias, sign, trace=False):
    """Run the kernel; returns (output, BassKernelResults)."""
    from concourse.bass_utils import run_bass_kernel_spmd

    if not trace:
        os.environ["BASS_NEVER_TRACE"] = "1"
    else:
        os.environ.pop("BASS_NEVER_TRACE", None)

    nc = _build()
    in_maps = _prep_inputs(x, weight, bias, sign)
    res = run_bass_kernel_spmd(
        nc,
        in_maps,
        core_ids=list(range(N_CORES)),
        trace=trace,
    )
    return _assemble(res.results), res


def kernel(x, weight, bias, sign):
    out, _ = run(x, weight, bias, sign, trace=False)
    return out


# revision 18
# speedup vs baseline: 1.0342x; 1.0342x over previous
# Trainium2 Bass kernel for nn_BinLinearEval:
#   out[b, o] = (round(x @ W.T + bias) * sign >= 0) ? 1.0 : 0.0
#
# Math folding (exact because bias is integer-valued and sign in {-1,+1}):
#   out = 1  iff  sign*(dot + bias) >= -0.5
#       = 1  iff  dot' >= thr_o      where dot' = x @ (sign.T*W).T  (W' still
#         ternary) and thr_o = -sign_o*bias_o - 0.5.
#
# Precision: x is shipped as an e4m3 hi + e4m3 residual*64 pair (2 B/elem)
# and BOTH passes run as fp8 DoubleRow matmuls. ~1700 threshold flips of
# 16.7M (rel err ~0.014 vs the 2e-2 gate).
#
# Measured facts this schedule is built on (NTFF traces):
#  - At 8-core load the chip sits in P0: PE clock ~2.0 GHz, so a DR FD=512
#    matmul stream paces at exactly 259 ns/MM (216 ns single-core). The
#    256-MM stream is a hard 66.3 us floor; LDWEIGHTS fully hides in the
#    pull-ahead window at any weight-reuse pattern, so no LDW amortization
#    is needed.
#  - The framework preamble ends ~6.4 us; first DMA bytes move ~8.7-9 us.
#    Both HWDGE rings share the 16 SDMA engines per 4KB packet, so each
#    ring sustains ~175 GB/s while both are busy (~350 aggregate = HBM cap).
#  - Receipts (sem>=16) land ~50 ns after transfer-done; what matters is
#    pure need-ordering of the two ring FIFOs.
# Schedule: warmup MMs run on memset tiles (no DMA dependency) from ~7 us
# so HAM un-throttles before real data lands; the first 3 groups and w8
# are split across both rings in need order; outs are merged per group
# (1 KB/partition) and alternate rings.

import os
from contextlib import ExitStack

import numpy as np
import ml_dtypes

BATCH, IN_F, OUT_F = 65536, 1024, 256
N_CORES = 8
B_CORE = BATCH // N_CORES  # 8192
P = 128
KC = IN_F // P             # 8 k-chunks of 128
NCH = KC                   # 8 DoubleRow chunk-steps: 4 hi + 4 lo, 256-contract each
OC = OUT_F // P            # 2 out-channel chunks
GRP = 512                  # batch tile (= max DR matmul moving dim / 2)
N_GROUPS = B_CORE // GRP   # 16
N_WARM = 10                # dummy MMs spanning ~4.3 us of PE-busy before data

_CACHE = {}


def _build():
    """Build (and cache) the Bass module. Returns the compiled nc."""
    if "nc" in _CACHE:
        return _CACHE["nc"]

    import concourse.bacc as bacc
    import concourse.mybir as mybir
    import concourse.tile as tile

    nc = bacc.Bacc(
        "TRN2",
        target_bir_lowering=False,
        debug=False,
        num_devices=N_CORES,
    )

    f32 = mybir.dt.float32
    f8 = mybir.dt.float8e4
    DR = mybir.MatmulPerfMode.DoubleRow

    # x8 chunk layout: [P, group, chunk(0:4 hi, 4:8 lo), j, GRP] where the
    # DoubleRow pair (chunk c, j) covers global k = (c%4)*256 + j*128 + p
    x8_d = nc.dram_tensor(
        "x8", [P, N_GROUPS, NCH, 2, GRP], f8, kind="ExternalInput"
    ).ap()
    # weights split by oc so each half is one contiguous 2KB/partition DMA
    w8_d = nc.dram_tensor("w8", [P, OC, NCH, 2, P], f8, kind="ExternalInput").ap()
    # thr replicated x128 on host: full 1KB/partition descriptors instead
    # of 8B ones (which waste ~3us of ring turns at packet round-robin)
    thr_d = nc.dram_tensor("thr", [P, OC, P], f32, kind="ExternalInput").ap()
    out_d = nc.dram_tensor(
        "out", [P, N_GROUPS, OC, GRP], f8, kind="ExternalOutput"
    ).ap()

    with tile.TileContext(nc) as tc, ExitStack() as ctx:
        const = ctx.enter_context(tc.tile_pool(name="const", bufs=1))
        io = ctx.enter_context(tc.tile_pool(name="io", bufs=1))
        outp = ctx.enter_context(tc.tile_pool(name="outp", bufs=1))
        psum = ctx.enter_context(tc.tile_pool(name="psum", bufs=8, space="PSUM"))

        w8_sb = const.tile([P, OC, NCH, 2, P], f8)
        thr_sb = const.tile([P, OC, P], f32)
        warm_x = const.tile([P, 2, GRP], f8)

        xt = {}
        for g in range(N_GROUPS):
            xt[g] = io.tile([P, NCH, 2, GRP], f8, name=f"x{g}", bufs=1)

        # warmup operand comes from one memset, not DMA, so the PE can
        # start burning its HAM ramp right after the preamble barrier
        nc.vector.memset(warm_x, 0.25)

        # ── DMA triggers ──
        # Both HWDGE rings share the 16 SDMA engines per-packet, ~185 GB/s
        # each while both are busy. Every group is split hi/lo across the
        # two rings in lockstep so group k completes ~2.7k us after the
        # first bytes — always ahead of the PE's 3.46 us/group consumption.
        # thr's 8-byte-per-partition descriptors would waste ring turns at
        # the worst time, so it rides the idle gpsimd SWDGE path instead.
        # Ring start times vary per core/run by +-1.5 us, so the items the
        # first matmuls need (w8oc0, g0hi, g0lo, g1hi) are themselves split
        # across BOTH rings — the stream start depends on max(ring starts)
        # + ~2.1 us instead of one ring delivering 0.75 MB alone.
        H = NCH // 2
        Q = NCH // 4
        # both w8 halves first, one per ring (the interleaved stream needs
        # oc0-c0 AND oc1-c0 immediately); then g0/g1 in quarters
        # alternating rings, g2+ as hi/lo halves in parallel
        engs = [nc.sync, nc.scalar]
        nc.sync.dma_start(out=w8_sb[:, 0], in_=w8_d[:, 0])
        nc.scalar.dma_start(out=w8_sb[:, 1], in_=w8_d[:, 1])
        # g0 per-chunk (8 x 128KB) so the interleaved stream's dribble is
        # one chunk deep; g1 in quarters
        for c in range(NCH):
            engs[c % 2].dma_start(out=xt[0][:, c], in_=x8_d[:, 0, c])
        for q in range(4):
            engs[(q + 1) % 2].dma_start(
                out=xt[1][:, 2 * q : 2 * q + 2], in_=x8_d[:, 1, 2 * q : 2 * q + 2]
            )
        nc.scalar.dma_start(out=thr_sb, in_=thr_d)
        for g in range(2, N_GROUPS):
            nc.sync.dma_start(out=xt[g][:, :H], in_=x8_d[:, g, :H])
            nc.scalar.dma_start(out=xt[g][:, H:], in_=x8_d[:, g, H:])

        # ── PE warmup: data-independent DR MMs at cold pace (~0.43-0.52
        # us each) spanning ~4.3 us so HAM reaches K=8/8 before the first
        # real matmul. psum never read; slots recycle into the pool.
        wps = [psum.tile([P, GRP], f32, name="ps") for _ in range(2)]
        for i in range(N_WARM):
            nc.tensor.matmul(
                wps[i % 2], warm_x[:, :, :P], warm_x,
                start=True, stop=True, perf_mode=DR,
            )

        # ── main stream: 16 groups x 8 chunk-steps x 2 oc-passes ──
        # The oc passes are interleaved per chunk so fresh-x demand is a
        # steady ~296 GB/s (one 128KB chunk per 2 MMs) instead of 2x-supply
        # bursts during each oc0 pass — the rings then never fall behind.
        # The LAST group runs its oc passes sequentially so the oc0
        # epilogue + out drain ~1.7 us before the end (shorter tail), with
        # its out split per-oc across the two then-idle rings.
        for g in range(N_GROUPS):
            ob = outp.tile([P, OC, GRP], f8, name=f"ob{g}", bufs=1)
            last = g == N_GROUPS - 1
            if not last:
                pss = [psum.tile([P, GRP], f32, name="ps") for _ in range(OC)]
                for c in range(NCH):
                    for oc in range(OC):
                        nc.tensor.matmul(
                            pss[oc],
                            w8_sb[:, oc, c],
                            xt[g][:, c],
                            start=(c == 0),
                            stop=(c == NCH - 1),
                            perf_mode=DR,
                        )
                for oc in range(OC):
                    nc.vector.tensor_scalar(
                        ob[:, oc],
                        pss[oc],
                        thr_sb[:, oc, :1],
                        None,
                        mybir.AluOpType.is_ge,
                    )
                eng = nc.sync if g % 2 else nc.scalar
                eng.dma_start(out=out_d[:, g], in_=ob)
            else:
                for oc in range(OC):
                    ps = psum.tile([P, GRP], f32, name="ps")
                    for c in range(NCH):
                        nc.tensor.matmul(
                            ps,
                            w8_sb[:, oc, c],
                            xt[g][:, c],
                            start=(c == 0),
                            stop=(c == NCH - 1),
                            perf_mode=DR,
                        )
                    nc.vector.tensor_scalar(
                        ob[:, oc],
                        ps,
                        thr_sb[:, oc, :1],
                        None,
                        mybir.AluOpType.is_ge,
                    )
                    eng = nc.sync if oc == 0 else nc.scalar
                    eng.dma_start(out=out_d[:, g, oc], in_=ob[:, oc])

    nc.compile()
    _CACHE["nc"] = nc
    return nc


def _prep_inputs(x, weight, bias, sign):
    """Host-side prep: fold sign into weights, build thresholds, split x into
    an e4m3 hi + e4m3 residual*64 pair in DoubleRow-interleaved layout."""
    f8np = ml_dtypes.float8_e4m3fn
    x = np.asarray(x, dtype=np.float32)
    weight = np.asarray(weight, dtype=np.float32)
    bias = np.asarray(bias, dtype=np.float32)
    sign = np.asarray(sign, dtype=np.float32).reshape(1, OUT_F)

    wp = sign.T * weight                      # [OUT_F, IN_F], ternary
    thr = (-sign[0] * bias - np.float32(0.5)).astype(np.float32)  # [OUT_F]
    thr2 = np.ascontiguousarray(
        np.repeat(thr.reshape(OC, P).T[:, :, None], P, axis=2)
    )  # [P, OC, P] replicated for full-size DMA descriptors

    # weights: [P, oc, chunk, j, 128]; chunks 0:4 = W' (ternary, exact in
    # e4m3), 4:8 = W'/64 (+-2^-6, exact in e4m3)
    wT = wp.T  # [IN_F, OUT_F]
    whi = wT.reshape(NCH // 2, 2, P, OUT_F).transpose(2, 0, 1, 3)
    wlo = (wT * np.float32(1.0 / 64.0)).reshape(NCH // 2, 2, P, OUT_F).transpose(
        2, 0, 1, 3
    )
    w8 = np.concatenate([whi, wlo], axis=1)   # [P, NCH, 2, OUT_F]
    w8 = np.ascontiguousarray(
        w8.reshape(P, NCH, 2, OC, P).transpose(0, 3, 1, 2, 4)
    ).astype(f8np)                            # [P, OC, NCH, 2, P]

    xhi8 = x.astype(f8np)
    xlo8 = ((x - xhi8.astype(np.float32)) * np.float32(64.0)).astype(f8np)

    in_maps = []
    for c in range(N_CORES):
        sl = slice(c * B_CORE, (c + 1) * B_CORE)
        hi = xhi8[sl].reshape(N_GROUPS, GRP, NCH // 2, 2, P).transpose(
            4, 0, 2, 3, 1
        )                                      # [P, g, 4, 2, GRP]
        lo = xlo8[sl].reshape(N_GROUPS, GRP, NCH // 2, 2, P).transpose(
            4, 0, 2, 3, 1
        )
        x8 = np.ascontiguousarray(np.concatenate([hi, lo], axis=2))
        in_maps.append({"x8": x8, "w8": w8, "thr": thr2})
    return in_maps


def _assemble(results):
    """[core][P, N_GROUPS, OC, GRP] fp8 -> [BATCH, OUT_F] fp32"""
    parts = []
    for r in results:
        a = (
            np.asarray(r["out"])
            .view(ml_dtypes.float8_e4m3fn)
            .astype(np.float32)
            .reshape(P, N_GROUPS, OC, GRP)
        )
        # out[b, o]: b = g*GRP + col, o = oc*P + p
        parts.append(
            a.transpose(1, 3, 2, 0).reshape(B_CORE, OUT_F)
        )
    return np.ascontiguousarray(np.concatenate(parts, axis=0))


def run(x, weight, bias, sign, trace=False):
    """Run the kernel; returns (output, BassKernelResults)."""
    from concourse.bass_utils import run_bass_kernel_spmd

    if not trace:
        os.environ["BASS_NEVER_TRACE"] = "1"
    else:
        os.environ.pop("BASS_NEVER_TRACE", None)

    nc = _build()
    in_maps = _prep_inputs(x, weight, bias, sign)
    res = run_bass_kernel_spmd(
        nc,
        in_maps,
        core_ids=list(range(N_CORES)),
        trace=trace,
    )
    return _assemble(res.results), res


def kernel(x, weight, bias, sign):
    out, _ = run(x, weight, bias, sign, trace=False)
    return out


# revision 20
# speedup vs baseline: 1.0438x; 1.0093x over previous
# Trainium2 Bass kernel for nn_BinLinearEval:
#   out[b, o] = (round(x @ W.T + bias) * sign >= 0) ? 1.0 : 0.0
#
# Math folding (exact because bias is integer-valued and sign in {-1,+1}):
#   out = 1  iff  sign*(dot + bias) >= -0.5
#       = 1  iff  dot' >= thr_o      where dot' = x @ (sign.T*W).T  (W' still
#         ternary) and thr_o = -sign_o*bias_o - 0.5.
#
# Precision: x is shipped as an e4m3 hi + e4m3 residual*64 pair (2 B/elem)
# and BOTH passes run as fp8 DoubleRow matmuls. ~1700 threshold flips of
# 16.7M (rel err ~0.014 vs the 2e-2 gate).
#
# Measured facts this schedule is built on (NTFF traces):
#  - At 8-core load the chip sits in P0: PE clock ~2.0 GHz, so a DR FD=512
#    matmul stream paces at exactly 259 ns/MM (216 ns single-core). The
#    256-MM stream is a hard 66.3 us floor; LDWEIGHTS fully hides in the
#    pull-ahead window at any weight-reuse pattern, so no LDW amortization
#    is needed.
#  - The framework preamble ends ~6.4 us; first DMA bytes move ~8.7-9 us.
#    Both HWDGE rings share the 16 SDMA engines per 4KB packet, so each
#    ring sustains ~175 GB/s while both are busy (~350 aggregate = HBM cap).
#  - Receipts (sem>=16) land ~50 ns after transfer-done; what matters is
#    pure need-ordering of the two ring FIFOs.
# Schedule: warmup MMs run on memset tiles (no DMA dependency) from ~7 us
# so HAM un-throttles before real data lands; the first 3 groups and w8
# are split across both rings in need order; outs are merged per group
# (1 KB/partition) and alternate rings.

import os
from contextlib import ExitStack

import numpy as np
import ml_dtypes

BATCH, IN_F, OUT_F = 65536, 1024, 256
N_CORES = 8
B_CORE = BATCH // N_CORES  # 8192
P = 128
KC = IN_F // P             # 8 k-chunks of 128
NCH = KC                   # 8 DoubleRow chunk-steps: 4 hi + 4 lo, 256-contract each
OC = OUT_F // P            # 2 out-channel chunks
GRP = 512                  # batch tile (= max DR matmul moving dim / 2)
N_GROUPS = B_CORE // GRP   # 16
N_WARM = 9                 # dummy MMs spanning ~4.3 us of PE-busy before data

_CACHE = {}


def _build():
    """Build (and cache) the Bass module. Returns the compiled nc."""
    if "nc" in _CACHE:
        return _CACHE["nc"]

    import concourse.bacc as bacc
    import concourse.mybir as mybir
    import concourse.tile as tile

    nc = bacc.Bacc(
        "TRN2",
        target_bir_lowering=False,
        debug=False,
        num_devices=N_CORES,
    )

    f32 = mybir.dt.float32
    f8 = mybir.dt.float8e4
    DR = mybir.MatmulPerfMode.DoubleRow

    # x8 chunk layout: [P, group, chunk(0:4 hi, 4:8 lo), j, GRP] where the
    # DoubleRow pair (chunk c, j) covers global k = (c%4)*256 + j*128 + p
    x8_d = nc.dram_tensor(
        "x8", [P, N_GROUPS, NCH, 2, GRP], f8, kind="ExternalInput"
    ).ap()
    # weights split by oc so each half is one contiguous 2KB/partition DMA
    w8_d = nc.dram_tensor("w8", [P, OC, NCH, 2, P], f8, kind="ExternalInput").ap()
    # thr replicated x128 on host: full 1KB/partition descriptors instead
    # of 8B ones (which waste ~3us of ring turns at packet round-robin)
    thr_d = nc.dram_tensor("thr", [P, OC, P], f32, kind="ExternalInput").ap()
    out_d = nc.dram_tensor(
        "out", [P, N_GROUPS, OC, GRP], f8, kind="ExternalOutput"
    ).ap()

    with tile.TileContext(nc) as tc, ExitStack() as ctx:
        const = ctx.enter_context(tc.tile_pool(name="const", bufs=1))
        io = ctx.enter_context(tc.tile_pool(name="io", bufs=1))
        outp = ctx.enter_context(tc.tile_pool(name="outp", bufs=1))
        psum = ctx.enter_context(tc.tile_pool(name="psum", bufs=8, space="PSUM"))

        w8_sb = const.tile([P, OC, NCH, 2, P], f8)
        thr_sb = const.tile([P, OC, P], f32)
        warm_x = const.tile([P, 2, GRP], f8)

        xt = {}
        for g in range(N_GROUPS):
            xt[g] = io.tile([P, NCH, 2, GRP], f8, name=f"x{g}", bufs=1)

        # warmup operand comes from one memset, not DMA, so the PE can
        # start burning its HAM ramp right after the preamble barrier
        nc.vector.memset(warm_x, 0.25)

        # ── DMA triggers ──
        # Both HWDGE rings share the 16 SDMA engines per-packet, ~185 GB/s
        # each while both are busy. Every group is split hi/lo across the
        # two rings in lockstep so group k completes ~2.7k us after the
        # first bytes — always ahead of the PE's 3.46 us/group consumption.
        # thr's 8-byte-per-partition descriptors would waste ring turns at
        # the worst time, so it rides the idle gpsimd SWDGE path instead.
        # Ring start times vary per core/run by +-1.5 us, so the items the
        # first matmuls need (w8oc0, g0hi, g0lo, g1hi) are themselves split
        # across BOTH rings — the stream start depends on max(ring starts)
        # + ~2.1 us instead of one ring delivering 0.75 MB alone.
        H = NCH // 2
        Q = NCH // 4
        # both w8 halves first, one per ring (the interleaved stream needs
        # oc0-c0 AND oc1-c0 immediately); then g0/g1 in quarters
        # alternating rings, g2+ as hi/lo halves in parallel
        # g0 in quarters (2KB descriptors, receipt per 2 chunks — fine
        # enough for the stream start without poisoning ring turn
        # efficiency); all other groups as hi/lo halves (4KB descriptors)
        engs = [nc.sync, nc.scalar]
        nc.sync.dma_start(out=w8_sb[:, 0], in_=w8_d[:, 0])
        nc.scalar.dma_start(out=w8_sb[:, 1], in_=w8_d[:, 1])
        for q in range(4):
            engs[q % 2].dma_start(
                out=xt[0][:, 2 * q : 2 * q + 2], in_=x8_d[:, 0, 2 * q : 2 * q + 2]
            )
        for g in range(1, N_GROUPS):
            nc.sync.dma_start(out=xt[g][:, :H], in_=x8_d[:, g, :H])
            nc.scalar.dma_start(out=xt[g][:, H:], in_=x8_d[:, g, H:])
            if g == 3:
                # thr needed only when g0's epilogue becomes psum-critical
                # (~25 us); late enough to stay off the critical supply path
                nc.scalar.dma_start(out=thr_sb, in_=thr_d)

        # ── PE warmup: data-independent DR MMs at cold pace (~0.43-0.52
        # us each) spanning ~4.3 us so HAM reaches K=8/8 before the first
        # real matmul. psum never read; slots recycle into the pool.
        wps = [psum.tile([P, GRP], f32, name="ps") for _ in range(2)]
        for i in range(N_WARM):
            nc.tensor.matmul(
                wps[i % 2], warm_x[:, :, :P], warm_x,
                start=True, stop=True, perf_mode=DR,
            )

        # ── main stream: 16 groups x 8 chunk-steps x 2 oc-passes ──
        # The oc passes are interleaved per chunk so fresh-x demand is a
        # steady ~296 GB/s (one 128KB chunk per 2 MMs) instead of 2x-supply
        # bursts during each oc0 pass — the rings then never fall behind.
        # The LAST group runs its oc passes sequentially so the oc0
        # epilogue + out drain ~1.7 us before the end (shorter tail), with
        # its out split per-oc across the two then-idle rings.
        for g in range(N_GROUPS):
            ob = outp.tile([P, OC, GRP], f8, name=f"ob{g}", bufs=1)
            last = g == N_GROUPS - 1
            if not last:
                pss = [psum.tile([P, GRP], f32, name="ps") for _ in range(OC)]
                for c in range(NCH):
                    for oc in range(OC):
                        nc.tensor.matmul(
                            pss[oc],
                            w8_sb[:, oc, c],
                            xt[g][:, c],
                            start=(c == 0),
                            stop=(c == NCH - 1),
                            perf_mode=DR,
                        )
                for oc in range(OC):
                    nc.vector.tensor_scalar(
                        ob[:, oc],
                        pss[oc],
                        thr_sb[:, oc, :1],
                        None,
                        mybir.AluOpType.is_ge,
                    )
                eng = nc.sync if g % 2 else nc.scalar
                eng.dma_start(out=out_d[:, g], in_=ob)
            else:
                for oc in range(OC):
                    ps = psum.tile([P, GRP], f32, name="ps")
                    for c in range(NCH):
                        nc.tensor.matmul(
                            ps,
                            w8_sb[:, oc, c],
                            xt[g][:, c],
                            start=(c == 0),
                            stop=(c == NCH - 1),
                            perf_mode=DR,
                        )
                    nc.vector.tensor_scalar(
                        ob[:, oc],
                        ps,
                        thr_sb[:, oc, :1],
                        None,
                        mybir.AluOpType.is_ge,
                    )
                    eng = nc.sync if oc == 0 else nc.scalar
                    eng.dma_start(out=out_d[:, g, oc], in_=ob[:, oc])

    nc.compile()
    _CACHE["nc"] = nc
    return nc


def _prep_inputs(x, weight, bias, sign):
    """Host-side prep: fold sign into weights, build thresholds, split x into
    an e4m3 hi + e4m3 residual*64 pair in DoubleRow-interleaved layout."""
    f8np = ml_dtypes.float8_e4m3fn
    x = np.asarray(x, dtype=np.float32)
    weight = np.asarray(weight, dtype=np.float32)
    bias = np.asarray(bias, dtype=np.float32)
    sign = np.asarray(sign, dtype=np.float32).reshape(1, OUT_F)

    wp = sign.T * weight                      # [OUT_F, IN_F], ternary
    thr = (-sign[0] * bias - np.float32(0.5)).astype(np.float32)  # [OUT_F]
    thr2 = np.ascontiguousarray(
        np.repeat(thr.reshape(OC, P).T[:, :, None], P, axis=2)
    )  # [P, OC, P] replicated for full-size DMA descriptors

    # weights: [P, oc, chunk, j, 128]; chunks 0:4 = W' (ternary, exact in
    # e4m3), 4:8 = W'/64 (+-2^-6, exact in e4m3)
    wT = wp.T  # [IN_F, OUT_F]
    whi = wT.reshape(NCH // 2, 2, P, OUT_F).transpose(2, 0, 1, 3)
    wlo = (wT * np.float32(1.0 / 64.0)).reshape(NCH // 2, 2, P, OUT_F).transpose(
        2, 0, 1, 3
    )
    w8 = np.concatenate([whi, wlo], axis=1)   # [P, NCH, 2, OUT_F]
    w8 = np.ascontiguousarray(
        w8.reshape(P, NCH, 2, OC, P).transpose(0, 3, 1, 2, 4)
    ).astype(f8np)                            # [P, OC, NCH, 2, P]

    xhi8 = x.astype(f8np)
    xlo8 = ((x - xhi8.astype(np.float32)) * np.float32(64.0)).astype(f8np)

    in_maps = []
    for c in range(N_CORES):
        sl = slice(c * B_CORE, (c + 1) * B_CORE)
        hi = xhi8[sl].reshape(N_GROUPS, GRP, NCH // 2, 2, P).transpose(
            4, 0, 2, 3, 1
        )                                      # [P, g, 4, 2, GRP]
        lo = xlo8[sl].reshape(N_GROUPS, GRP, NCH // 2, 2, P).transpose(
            4, 0, 2, 3, 1
        )
        x8 = np.ascontiguousarray(np.concatenate([hi, lo], axis=2))
        in_maps.append({"x8": x8, "w8": w8, "thr": thr2})
    return in_maps


def _assemble(results):
    """[core][P, N_GROUPS, OC, GRP] fp8 -> [BATCH, OUT_F] fp32"""
    parts = []
    for r in results:
        a = (
            np.asarray(r["out"])
            .view(ml_dtypes.float8_e4m3fn)
            .astype(np.float32)
            .reshape(P, N_GROUPS, OC, GRP)
        )
        # out[b, o]: b = g*GRP + col, o = oc*P + p
        parts.append(
            a.transpose(1, 3, 2, 0).reshape(B_CORE, OUT_F)
        )
    return np.ascontiguousarray(np.concatenate(parts, axis=0))


def run(x, weight, bias, sign, trace=False):
    """Run the kernel; returns (output, BassKernelResults)."""
    from concourse.bass_utils import run_bass_kernel_spmd

    if not trace:
        os.environ["BASS_NEVER_TRACE"] = "1"
    else:
        os.environ.pop("BASS_NEVER_TRACE", None)

    nc = _build()
    in_maps = _prep_inputs(x, weight, bias, sign)
    res = run_bass_kernel_spmd(
        nc,
        in_maps,
        core_ids=list(range(N_CORES)),
        trace=trace,
    )
    return _assemble(res.results), res


def kernel(x, weight, bias, sign):
    out, _ = run(x, weight, bias, sign, trace=False)
    return out
